# revision 16
# baseline (speedup 1.0000x reference)
"""Multi-head causal self-attention (B=2, T=2048, C=1024, H=16) on 8 trn2 cores.

Sharding: data-parallel over batch (2) x tensor-parallel over heads (4 groups
of 4 heads). Core c handles batch b=c//4, head group g=c%4.

v3: per-q-block software pipeline. QKV GEMM chunks for q-block qb are emitted
right before qb's attention so the Act engine (exp, the attention-phase
bottleneck) starts early and overlaps the PE-bound GEMM work. Collectives are
serialized through SP DMA-queue order (out_dma of RS(i) precedes the partial
writes feeding RS(i+1)). Exps run on paired 1024-wide PSUM tiles to halve
Act init overhead. Biases for V/proj are added as rank-1 PE accumulations so
all evacuations are plain copies, balanced across DVE and Act.
"""

import os

import numpy as np

import concourse.bacc as bacc
import concourse.mybir as mybir
import concourse.tile as tile
from concourse.bass_utils import run_bass_kernel_spmd

DEBUG = bool(int(os.environ.get("KERNEL_DEBUG", "0")))

F32 = mybir.dt.float32
F16 = mybir.dt.float16

B, T, C, H = 2, 2048, 1024, 16
HPC = 4                # heads per core
HD = 64                # head dim
CG = HPC * 3 * HD      # 768 qkv cols per core
PD = HPC * HD          # 256 proj rows per core
TT = T // 128          # 16 t tiles
KC = C // 128          # 8 contraction tiles
QB = 4                 # q blocks of 512
N_CORES = 8


def _build():
    nc = bacc.Bacc(None, target_bir_lowering=False)

    x_in = nc.dram_tensor("x", [T, C], F32, kind="ExternalInput")
    wqkv_in = nc.dram_tensor("wqkv", [C, CG], F32, kind="ExternalInput")
    bqkv_in = nc.dram_tensor("bqkv", [1, CG], F32, kind="ExternalInput")
    wproj_in = nc.dram_tensor("wproj", [PD, C], F32, kind="ExternalInput")
    bproj_in = nc.dram_tensor("bproj", [1, C], F32, kind="ExternalInput")
    out_part = nc.dram_tensor("out_part", [T // 4, C], F16, kind="ExternalOutput")

    partial_d = nc.dram_tensor("partial_d", [T, C], F16)
    rsout_d = [nc.dram_tensor(f"rsout_d{i}", [T // 16, C], F16) for i in range(QB)]

    with tile.TileContext(nc) as tc:
        with (
            tc.tile_pool(name="cpool", bufs=1) as cpool,
            tc.tile_pool(name="main", bufs=1) as main,
            tc.tile_pool(name="stage", bufs=1) as stage,
        ):
            # -------- x DMAs first (HWDGE via SP), graduated chunks --------
            CHUNK_TILES = [[0], [1], [2, 3], [4, 5], [6, 7], [8, 9],
                           [10, 11], [12, 13], [14, 15]]
            QB_CHUNKS = [[0, 1, 2], [3, 4], [5, 6], [7, 8]]
            x_r = x_in.rearrange("(t p) c -> p t c", p=128)
            x_q = []
            for ci, tls in enumerate(CHUNK_TILES):
                xq_t = stage.tile(
                    [128, len(tls) * C], F32, tag="xq", bufs=3, name=f"xq{ci}"
                )
                x_q.append(xq_t)
                nc.sync.dma_start(
                    xq_t[:].rearrange("p (a c) -> p a c", a=len(tls)),
                    x_r[:, tls[0] : tls[-1] + 1, :],
                )

            # -------- constants + weight DMAs (Pool/SWDGE for casts) --------
            ident32 = cpool.tile([128, 128], F32)
            nc.gpsimd.memset(ident32[:], 0.0)
            nc.gpsimd.affine_select(
                out=ident32[:], in_=ident32[:],
                compare_op=mybir.AluOpType.not_equal, fill=1.0,
                base=0, pattern=[[-1, 128]], channel_multiplier=1,
            )
            # S^T diag mask: keep (1) where q >= k (x=k part, y=q free)
            mask_t = cpool.tile([128, 128], F16)
            nc.gpsimd.memset(mask_t[:], 1.0)
            nc.gpsimd.affine_select(
                out=mask_t[:], in_=mask_t[:],
                compare_op=mybir.AluOpType.is_ge, fill=0.0,
                base=0, pattern=[[1, 128]], channel_multiplier=-1,
            )
            ones_row = cpool.tile([1, 128], F16)
            nc.vector.memset(ones_row[:], 1.0)
            scrap = cpool.tile([1, 8], F16)
            nc.vector.memset(scrap[:], 0.0)
            scrap2 = cpool.tile([1, 8], F16)
            # prime both activation tables before attention needs them
            nc.scalar.activation(
                scrap2[:], scrap[:], mybir.ActivationFunctionType.Exp, scale=0.125
            )
            nc.scalar.activation(
                scrap2[:], scrap[:], mybir.ActivationFunctionType.Copy
            )
            nc.scalar.activation(
                scrap2[:], scrap[:], mybir.ActivationFunctionType.Identity
            )

            wq16 = main.tile([128, KC * CG], F16)
            nc.gpsimd.dma_start(
                wq16[:].rearrange("p (k c) -> p k c", k=KC),
                wqkv_in.rearrange("(k p) c -> p k c", p=128),
            )
            vb_row = cpool.tile([1, HPC * HD], F16)
            nc.gpsimd.dma_start(vb_row[:], bqkv_in[0:1, 512:768])
            # qk bias vectors (128,1): [q01, q23, k01, k23] (host pre-permuted)
            qk_bias = cpool.tile([128, 4], F32)
            nc.gpsimd.dma_start(
                qk_bias[:], bqkv_in[0:1, 0:512].rearrange("o (i p) -> p (o i)", p=128)
            )
            pb_row = cpool.tile([1, C], F16)
            nc.gpsimd.dma_start(pb_row[:], bproj_in[0:1, :])
            # proj weights: head-pair stacked (128 chans each)
            wp2t = main.tile([128, 2 * C], F16)
            nc.gpsimd.dma_start(
                wp2t[:].rearrange("p (k c) -> p k c", k=2),
                wproj_in.rearrange("(k p) c -> p k c", p=128),
            )
            wp2 = [wp2t[:, 0:C], wp2t[:, C : 2 * C]]

            # -------- persistent tensors --------
            xT = main.tile([128, KC * T], F16)
            xT_v = xT[:].rearrange("p (k t) -> p k t", t=T)
            qkT = main.tile([128, 4 * T], F16)          # [Q01; Q23; K01; K23]
            v_aug = main.tile([128, TT * HPC * 65], F16)
            oT2 = [main.tile([128, T], F16, name=f"oT2_{i}") for i in range(2)]
            vbias_rep = main.tile([128, HPC * HD], F16)
            pbias_rep = main.tile([128, C], F16)

            nc.vector.memset(v_aug[:], 1.0)  # ones cols for rowsums

            dbg = {}
            if DEBUG:
                dbg["qkT"] = nc.dram_tensor("dbg_qkT", [128, 4 * T], F32, kind="ExternalOutput")
                dbg["v_aug"] = nc.dram_tensor("dbg_v_aug", [128, TT * HPC * 65], F32, kind="ExternalOutput")
                dbg["oT2"] = nc.dram_tensor("dbg_oT2", [128, 2 * T], F32, kind="ExternalOutput")
                dbg["partial"] = nc.dram_tensor("dbg_partial", [T, C], F32, kind="ExternalOutput")

            part_r = partial_d.rearrange("(a p) c -> p a c", p=128)

            with tc.tile_pool(name="psK", bufs=1, space="PSUM") as pK:
                ot = {}  # l -> live (65, 512) PSUM tile for current qb

                # ---------------- emission helpers ----------------
                def transpose_granule(ci, half, tt, g2):
                    xt_ps = pK.tile([128, 512], F32, tag="big", bufs=5)
                    for j in range(4):
                        kc = g2 * 4 + j
                        nc.tensor.transpose(
                            xt_ps[:, j * 128 : (j + 1) * 128],
                            x_q[ci][:, half * C + kc * 128 : half * C + (kc + 1) * 128],
                            ident32[:],
                        )
                    dst = xT_v[:, g2 * 4 : (g2 + 1) * 4, tt * 128 : (tt + 1) * 128]
                    src = xt_ps[:].rearrange("p (k t) -> p k t", t=128)
                    if (tt + g2) % 2 == 0:
                        nc.scalar.activation(
                            dst, src, mybir.ActivationFunctionType.Copy
                        )
                    else:
                        nc.vector.tensor_copy(dst, src)

                def v_chunk(tt):
                    psb = pK.tile([128, 512], F32, tag="big", bufs=5)
                    ps = psb[:, 0:256]
                    for kc in range(KC):
                        nc.tensor.matmul(
                            ps[:],
                            xT[:, kc * T + tt * 128 : kc * T + (tt + 1) * 128],
                            wq16[:, kc * CG + 512 : kc * CG + 768],
                            start=(kc == 0),
                            stop=False,
                        )
                    nc.tensor.matmul(
                        ps[:], ones_row[:, :], vb_row[:], start=False, stop=True
                    )
                    vt = v_aug[:, tt * HPC * 65 : (tt + 1) * HPC * 65].rearrange(
                        "p (h c) -> p h c", c=65
                    )[:, :, 0:64]
                    nc.scalar.activation(
                        vt, ps[:].rearrange("p (h c) -> p h c", c=64),
                        mybir.ActivationFunctionType.Copy,
                    )

                def qk_chunk(i, tch):
                    ps2 = pK.tile([128, 512], F32, tag="big", bufs=5)
                    for kc in range(KC):
                        nc.tensor.matmul(
                            ps2[:],
                            wq16[:, kc * CG + i * 128 : kc * CG + (i + 1) * 128],
                            xT[:, kc * T + tch * 512 : kc * T + (tch + 1) * 512],
                            start=(kc == 0),
                            stop=(kc == KC - 1),
                        )
                    if (i + tch) % 2 == 0:
                        nc.scalar.activation(
                            qkT[:, i * T + tch * 512 : i * T + (tch + 1) * 512],
                            ps2[:],
                            mybir.ActivationFunctionType.Identity,
                            bias=qk_bias[:, i : i + 1],
                        )
                    else:
                        nc.vector.tensor_scalar_add(
                            qkT[:, i * T + tch * 512 : i * T + (tch + 1) * 512],
                            ps2[:],
                            qk_bias[:, i : i + 1],
                        )

                def attn(l, qb, feed):
                    p0 = 64 * (l % 2)
                    qT = qkT[p0 : p0 + 64, (l // 2) * T + qb * 512 : (l // 2) * T + (qb + 1) * 512]
                    kT = qkT[p0 : p0 + 64, (2 + l // 2) * T : (3 + l // 2) * T]
                    o = pK.tile([65, 512], F32, tag="ot", bufs=3)
                    ot[l] = o
                    # diagonal band first: its S->exp->mask->PV latency chain is
                    # then covered by the queued full-tile matmuls behind it
                    first = [True]

                    def flag():
                        f = first[0]
                        first[0] = False
                        return f

                    for j in range(4):
                        kj = 4 * qb + j
                        off = j * 128
                        st = pK.tile([128, 512], F32, tag="big", bufs=5)
                        nc.tensor.matmul(
                            st[:, off:512],
                            kT[:, kj * 128 : (kj + 1) * 128],
                            qT[:, off:512],
                            start=True,
                            stop=True,
                        )
                        pt = stage.tile([128, 512], F16, tag="pt", bufs=6)
                        nc.scalar.activation(
                            pt[:, off:512], st[:, off:512],
                            mybir.ActivationFunctionType.Exp,
                            scale=0.125,
                        )
                        nc.gpsimd.tensor_mul(
                            pt[:, off : off + 128], pt[:, off : off + 128], mask_t[:]
                        )
                        vv = v_aug[:, kj * HPC * 65 + l * 65 : kj * HPC * 65 + (l + 1) * 65]
                        if off + 128 < 512:
                            nc.tensor.matmul(
                                o[:, off + 128 : 512], vv, pt[:, off + 128 : 512],
                                start=flag(), stop=False,
                            )
                        nc.tensor.matmul(
                            o[:, off : off + 128], vv, pt[:, off : off + 128],
                            start=flag() if off == 0 else False,
                            stop=(qb == 0),
                        )
                        feed()
                    for kj in range(4 * qb):
                        st = pK.tile([128, 512], F32, tag="big", bufs=5)
                        nc.tensor.matmul(
                            st[:],
                            kT[:, kj * 128 : (kj + 1) * 128],
                            qT[:],
                            start=True,
                            stop=True,
                        )
                        pt = stage.tile([128, 512], F16, tag="pt", bufs=6)
                        nc.scalar.activation(
                            pt[:], st[:],
                            mybir.ActivationFunctionType.Exp,
                            scale=0.125,
                        )
                        vv = v_aug[:, kj * HPC * 65 + l * 65 : kj * HPC * 65 + (l + 1) * 65]
                        nc.tensor.matmul(
                            o[:], vv, pt[:],
                            start=False,
                            stop=(kj == 4 * qb - 1),
                        )
                        feed()

                def norm(pair, qb, bc_act=False):
                    # heads 2*pair, 2*pair+1 -> oT2[pair] cols [qb*512, +512)
                    l0, l1 = 2 * pair, 2 * pair + 1
                    bc = pK.tile([128, 512], F32, tag="big", bufs=5)
                    for k, l in enumerate((l0, l1)):
                        rs_sb = stage.tile([1, 512], F32, tag=f"rs{k}", bufs=2, name=f"rs_{k}")
                        nc.vector.tensor_copy(rs_sb[:], ot[l][64:65, :])
                        rc = stage.tile([1, 512], F32, tag=f"rc{k}", bufs=2, name=f"rc_{k}")
                        nc.vector.reciprocal_approx_fast(rc[:], rs_sb[:])
                        rc16 = stage.tile([1, 512], F16, tag=f"rc16{k}", bufs=2, name=f"rc16_{k}")
                        nc.vector.tensor_copy(rc16[:], rc[:])
                        nc.tensor.matmul(
                            bc[k * 64 : (k + 1) * 64, :],
                            ones_row[:, 0:64],
                            rc16[:],
                            start=True,
                            stop=True,
                        )
                    bc_sb = stage.tile([128, 512], F16, tag="bcs", bufs=2)
                    if bc_act:
                        nc.scalar.activation(
                            bc_sb[:], bc[:], mybir.ActivationFunctionType.Copy
                        )
                    else:
                        nc.vector.tensor_copy(bc_sb[:], bc[:])
                    dst = oT2[pair][:, qb * 512 : (qb + 1) * 512]
                    nc.vector.tensor_mul(dst[0:64, :], ot[l0][0:64, :], bc_sb[0:64, :])
                    nc.vector.tensor_mul(dst[64:128, :], ot[l1][0:64, :], bc_sb[64:128, :])

                def proj_half(qb, hf, split_dma=False):
                    part2 = stage.tile([128, 2 * C], F16, tag="part", bufs=2)
                    for jj, j in enumerate((2 * hf, 2 * hf + 1)):
                        tt = qb * 4 + j
                        for nch in range(2):
                            pp = pK.tile([128, 512], F32, tag="big", bufs=5)
                            for hp in range(2):
                                nc.tensor.matmul(
                                    pp[:],
                                    oT2[hp][:, tt * 128 : (tt + 1) * 128],
                                    wp2[hp][:, nch * 512 : (nch + 1) * 512],
                                    start=(hp == 0),
                                    stop=(hp == 1),
                                )
                            nc.vector.scalar_tensor_tensor(
                                out=part2[:, jj * C + nch * 512 : jj * C + (nch + 1) * 512],
                                in0=pp[:],
                                scalar=1.0,
                                in1=pbias_rep[:, nch * 512 : (nch + 1) * 512],
                                op0=mybir.AluOpType.mult,
                                op1=mybir.AluOpType.add,
                            )
                    tt0 = qb * 4 + 2 * hf
                    if split_dma:
                        for jj in range(2):
                            nc.sync.dma_start(
                                part_r[:, tt0 + jj : tt0 + jj + 1, :],
                                part2[:, jj * C : (jj + 1) * C].rearrange(
                                    "p (a c) -> p a c", a=1
                                ),
                            )
                    else:
                        nc.sync.dma_start(
                            part_r[:, tt0 : tt0 + 2, :],
                            part2[:].rearrange("p (a c) -> p a c", a=2),
                        )

                def rs_coll(qb):
                    nc.gpsimd.collective_compute(
                        "ReduceScatter",
                        mybir.AluOpType.add,
                        replica_groups=[[0, 1, 2, 3], [4, 5, 6, 7]],
                        ins=[partial_d[qb * 512 : (qb + 1) * 512, :]],
                        outs=[rsout_d[qb][:]],
                    )

                def out_dma(qb):
                    # on SP: also serializes collectives — partial writes for
                    # RS(qb+1) queue behind this wait on RS(qb) completion
                    nc.sync.dma_start(
                        out_part[qb * 128 : (qb + 1) * 128, :], rsout_d[qb][:]
                    )

                # ---------------- emission schedule ----------------
                def gemm_items(qb, defer_v=False):
                    items = []
                    deferred = []
                    for ci in QB_CHUNKS[qb]:
                        for half, tt in enumerate(CHUNK_TILES[ci]):
                            for g2 in range(2):
                                items.append(
                                    lambda ci=ci, half=half, tt=tt, g2=g2:
                                        transpose_granule(ci, half, tt, g2)
                                )
                    for tt in range(4 * qb, 4 * qb + 4):
                        (deferred if defer_v else items).append(
                            lambda tt=tt: v_chunk(tt)
                        )
                    for i in (0, 2, 1, 3):
                        items.append(lambda i=i, qb=qb: qk_chunk(i, qb))
                    return items, deferred

                def make_feeder(items, nsteps):
                    state = {"credit": 0.0, "idx": 0}
                    rate = len(items) / max(1, nsteps)

                    def feed():
                        state["credit"] += rate
                        while state["credit"] >= 1.0 and state["idx"] < len(items):
                            items[state["idx"]]()
                            state["idx"] += 1
                            state["credit"] -= 1.0
                    def drain():
                        while state["idx"] < len(items):
                            items[state["idx"]]()
                            state["idx"] += 1
                    return feed, drain

                for ci in QB_CHUNKS[0]:
                    transpose_chunk_items = [
                        (lambda ci=ci, half=half, tt=tt, g2=g2:
                            transpose_granule(ci, half, tt, g2))
                        for half, tt in enumerate(CHUNK_TILES[ci])
                        for g2 in range(2)
                    ]
                    for it in transpose_chunk_items:
                        it()
                bias_reps()
                for tt in range(4):
                    v_chunk(tt)
                for i in (0, 2, 1, 3):
                    qk_chunk(i, 0)
                deferred_v = []
                for qb in range(QB):
                    if qb + 1 < QB:
                        items, deferred_v = gemm_items(qb + 1, defer_v=(qb + 1 == 3))
                    else:
                        items = deferred_v
                    # deferred V items must finish within head 0's full-tile
                    # steps: attn(0)'s diagonal band reads v_aug written by them
                    nsteps = 4 * qb if qb + 1 == QB else 4 * (4 * qb + 4)
                    feed, drain = make_feeder(items, nsteps)
                    # head 0 of the last block must run its diagonal band last
                    # (deferred V chunks are fed during its full tiles); other
                    # heads take the diag band first to keep its latency chain
                    # off the critical path
                    dfirst = qb != 0
                    attn(0, qb, feed, qb not in (0, QB - 1))
                    if qb > 0:
                        norm(1, qb - 1)
                        if qb > 1:
                            out_dma(qb - 2)
                        proj_half(qb - 1, 0)
                    attn(1, qb, feed, dfirst)
                    if qb > 0:
                        proj_half(qb - 1, 1)
                        rs_coll(qb - 1)
                    attn(2, qb, feed, dfirst)
                    norm(0, qb)
                    attn(3, qb, feed, dfirst)
                    drain()
                norm(1, QB - 1, bc_act=True)
                out_dma(QB - 2)
                proj_half(QB - 1, 0)
                proj_half(QB - 1, 1, split_dma=True)
                rs_coll(QB - 1)
                out_dma(QB - 1)

                if DEBUG:
                    nc.gpsimd.dma_start(dbg["qkT"][:], qkT[:])
                    nc.gpsimd.dma_start(dbg["v_aug"][:], v_aug[:])
                    nc.gpsimd.dma_start(dbg["oT2"][:, 0:T], oT2[0][:])
                    nc.gpsimd.dma_start(dbg["oT2"][:, T : 2 * T], oT2[1][:])
                    nc.gpsimd.dma_start(dbg["partial"][:], partial_d[:])

    nc.finalize()
    return nc


_NC = None


def _get_nc():
    global _NC
    if _NC is None:
        _NC = _build()
    return _NC


def _make_in_maps(x, Wqkv, bqkv, Wproj, bproj):
    x = np.asarray(x, dtype=np.float32)
    Wqkv = np.asarray(Wqkv, dtype=np.float32)
    bqkv = np.asarray(bqkv, dtype=np.float32)
    Wproj = np.asarray(Wproj, dtype=np.float32)
    bproj = np.asarray(bproj, dtype=np.float32)
    zeros_c = np.zeros((1, C), np.float32)

    def perm_qkv(w):
        # (..., h*192 + t*64 + c) -> (..., t*256 + h*64 + c)
        s = w.shape[:-1]
        return np.ascontiguousarray(
            w.reshape(*s, HPC, 3, HD).swapaxes(-3, -2).reshape(*s, CG)
        )

    in_maps = []
    for c in range(N_CORES):
        b, g = divmod(c, 4)
        in_maps.append(
            {
                "x": np.ascontiguousarray(x[b]),
                "wqkv": perm_qkv(Wqkv[:, g * CG : (g + 1) * CG]),
                "bqkv": perm_qkv(bqkv[g * CG : (g + 1) * CG]).reshape(1, CG),
                "wproj": np.ascontiguousarray(Wproj[g * PD : (g + 1) * PD, :]),
                "bproj": bproj.reshape(1, C) if g == 0 else zeros_c,
            }
        )
    return in_maps


def _run(in_maps, trace=False):
    nc = _get_nc()
    return run_bass_kernel_spmd(nc, in_maps, list(range(N_CORES)), trace=trace)


def kernel(x, Wqkv, bqkv, Wproj, bproj):
    in_maps = _make_in_maps(x, Wqkv, bqkv, Wproj, bproj)
    res = _run(in_maps)
    out = np.empty((B, T, C), np.float32)
    for c in range(N_CORES):
        b, g = divmod(c, 4)
        op = np.asarray(res.results[c]["out_part"], dtype=np.float32)
        for qb in range(QB):
            out[b, qb * 512 + g * 128 : qb * 512 + (g + 1) * 128, :] = op[
                qb * 128 : (qb + 1) * 128
            ]
    return out


# revision 17
# speedup vs baseline: 1.0244x; 1.0244x over previous
"""Multi-head causal self-attention (B=2, T=2048, C=1024, H=16) on 8 trn2 cores.

Sharding: data-parallel over batch (2) x tensor-parallel over heads (4 groups
of 4 heads). Core c handles batch b=c//4, head group g=c%4.

v3: per-q-block software pipeline. QKV GEMM chunks for q-block qb are emitted
right before qb's attention so the Act engine (exp, the attention-phase
bottleneck) starts early and overlaps the PE-bound GEMM work. Collectives are
serialized through SP DMA-queue order (out_dma of RS(i) precedes the partial
writes feeding RS(i+1)). Exps run on paired 1024-wide PSUM tiles to halve
Act init overhead. Biases for V/proj are added as rank-1 PE accumulations so
all evacuations are plain copies, balanced across DVE and Act.
"""

import os

import numpy as np

import concourse.bacc as bacc
import concourse.mybir as mybir
import concourse.tile as tile
from concourse.bass_utils import run_bass_kernel_spmd

DEBUG = bool(int(os.environ.get("KERNEL_DEBUG", "0")))

F32 = mybir.dt.float32
F16 = mybir.dt.float16

B, T, C, H = 2, 2048, 1024, 16
HPC = 4                # heads per core
HD = 64                # head dim
CG = HPC * 3 * HD      # 768 qkv cols per core
PD = HPC * HD          # 256 proj rows per core
TT = T // 128          # 16 t tiles
KC = C // 128          # 8 contraction tiles
QB = 4                 # q blocks of 512
N_CORES = 8


def _build():
    nc = bacc.Bacc(None, target_bir_lowering=False)

    x_in = nc.dram_tensor("x", [T, C], F32, kind="ExternalInput")
    wqkv_in = nc.dram_tensor("wqkv", [C, CG], F32, kind="ExternalInput")
    bqkv_in = nc.dram_tensor("bqkv", [1, CG], F32, kind="ExternalInput")
    wproj_in = nc.dram_tensor("wproj", [PD, C], F32, kind="ExternalInput")
    bproj_in = nc.dram_tensor("bproj", [1, C], F32, kind="ExternalInput")
    out_part = nc.dram_tensor("out_part", [T // 4, C], F16, kind="ExternalOutput")

    partial_d = nc.dram_tensor("partial_d", [T, C], F16)
    rsout_d = [nc.dram_tensor(f"rsout_d{i}", [T // 16, C], F16) for i in range(QB)]

    with tile.TileContext(nc) as tc:
        with (
            tc.tile_pool(name="cpool", bufs=1) as cpool,
            tc.tile_pool(name="main", bufs=1) as main,
            tc.tile_pool(name="stage", bufs=1) as stage,
        ):
            # -------- x DMAs first (HWDGE via SP), graduated chunks --------
            CHUNK_TILES = [[0], [1], [2, 3], [4, 5], [6, 7], [8, 9],
                           [10, 11], [12, 13], [14, 15]]
            QB_CHUNKS = [[0, 1, 2], [3, 4], [5, 6], [7, 8]]
            x_r = x_in.rearrange("(t p) c -> p t c", p=128)
            x_q = []
            for ci, tls in enumerate(CHUNK_TILES):
                xq_t = stage.tile(
                    [128, len(tls) * C], F32, tag="xq", bufs=3, name=f"xq{ci}"
                )
                x_q.append(xq_t)
                nc.sync.dma_start(
                    xq_t[:].rearrange("p (a c) -> p a c", a=len(tls)),
                    x_r[:, tls[0] : tls[-1] + 1, :],
                )

            # -------- constants + weight DMAs (Pool/SWDGE for casts) --------
            ident32 = cpool.tile([128, 128], F32)
            nc.gpsimd.memset(ident32[:], 0.0)
            nc.gpsimd.affine_select(
                out=ident32[:], in_=ident32[:],
                compare_op=mybir.AluOpType.not_equal, fill=1.0,
                base=0, pattern=[[-1, 128]], channel_multiplier=1,
            )
            # S^T diag mask: keep (1) where q >= k (x=k part, y=q free)
            mask_t = cpool.tile([128, 128], F16)
            nc.gpsimd.memset(mask_t[:], 1.0)
            nc.gpsimd.affine_select(
                out=mask_t[:], in_=mask_t[:],
                compare_op=mybir.AluOpType.is_ge, fill=0.0,
                base=0, pattern=[[1, 128]], channel_multiplier=-1,
            )
            ones_row = cpool.tile([1, 128], F16)
            nc.vector.memset(ones_row[:], 1.0)
            scrap = cpool.tile([1, 8], F16)
            nc.vector.memset(scrap[:], 0.0)
            scrap2 = cpool.tile([1, 8], F16)
            # prime both activation tables before attention needs them
            nc.scalar.activation(
                scrap2[:], scrap[:], mybir.ActivationFunctionType.Exp, scale=0.125
            )
            nc.scalar.activation(
                scrap2[:], scrap[:], mybir.ActivationFunctionType.Copy
            )
            nc.scalar.activation(
                scrap2[:], scrap[:], mybir.ActivationFunctionType.Identity
            )

            wq16 = main.tile([128, KC * CG], F16)
            nc.gpsimd.dma_start(
                wq16[:].rearrange("p (k c) -> p k c", k=KC),
                wqkv_in.rearrange("(k p) c -> p k c", p=128),
            )
            vb_row = cpool.tile([1, HPC * HD], F16)
            nc.gpsimd.dma_start(vb_row[:], bqkv_in[0:1, 512:768])
            # qk bias vectors (128,1): [q01, q23, k01, k23] (host pre-permuted)
            qk_bias = cpool.tile([128, 4], F32)
            nc.gpsimd.dma_start(
                qk_bias[:], bqkv_in[0:1, 0:512].rearrange("o (i p) -> p (o i)", p=128)
            )
            pb_row = cpool.tile([1, C], F16)
            nc.gpsimd.dma_start(pb_row[:], bproj_in[0:1, :])
            # proj weights: head-pair stacked (128 chans each)
            wp2t = main.tile([128, 2 * C], F16)
            nc.gpsimd.dma_start(
                wp2t[:].rearrange("p (k c) -> p k c", k=2),
                wproj_in.rearrange("(k p) c -> p k c", p=128),
            )
            wp2 = [wp2t[:, 0:C], wp2t[:, C : 2 * C]]

            # -------- persistent tensors --------
            xT = main.tile([128, KC * T], F16)
            xT_v = xT[:].rearrange("p (k t) -> p k t", t=T)
            qkT = main.tile([128, 4 * T], F16)          # [Q01; Q23; K01; K23]
            v_aug = main.tile([128, TT * HPC * 65], F16)
            oT2 = [main.tile([128, T], F16, name=f"oT2_{i}") for i in range(2)]
            vbias_rep = main.tile([128, HPC * HD], F16)
            pbias_rep = main.tile([128, C], F16)

            nc.vector.memset(v_aug[:], 1.0)  # ones cols for rowsums

            dbg = {}
            if DEBUG:
                dbg["qkT"] = nc.dram_tensor("dbg_qkT", [128, 4 * T], F32, kind="ExternalOutput")
                dbg["v_aug"] = nc.dram_tensor("dbg_v_aug", [128, TT * HPC * 65], F32, kind="ExternalOutput")
                dbg["oT2"] = nc.dram_tensor("dbg_oT2", [128, 2 * T], F32, kind="ExternalOutput")
                dbg["partial"] = nc.dram_tensor("dbg_partial", [T, C], F32, kind="ExternalOutput")

            part_r = partial_d.rearrange("(a p) c -> p a c", p=128)

            with tc.tile_pool(name="psK", bufs=1, space="PSUM") as pK:
                ot = {}  # l -> live (65, 512) PSUM tile for current qb

                # ---------------- emission helpers ----------------
                def transpose_granule(ci, half, tt, g2):
                    xt_ps = pK.tile([128, 512], F32, tag="big", bufs=5)
                    for j in range(4):
                        kc = g2 * 4 + j
                        nc.tensor.transpose(
                            xt_ps[:, j * 128 : (j + 1) * 128],
                            x_q[ci][:, half * C + kc * 128 : half * C + (kc + 1) * 128],
                            ident32[:],
                        )
                    dst = xT_v[:, g2 * 4 : (g2 + 1) * 4, tt * 128 : (tt + 1) * 128]
                    src = xt_ps[:].rearrange("p (k t) -> p k t", t=128)
                    if (tt + g2) % 2 == 0:
                        nc.scalar.activation(
                            dst, src, mybir.ActivationFunctionType.Copy
                        )
                    else:
                        nc.vector.tensor_copy(dst, src)

                def v_chunk(tt):
                    psb = pK.tile([128, 512], F32, tag="big", bufs=5)
                    ps = psb[:, 0:256]
                    for kc in range(KC):
                        nc.tensor.matmul(
                            ps[:],
                            xT[:, kc * T + tt * 128 : kc * T + (tt + 1) * 128],
                            wq16[:, kc * CG + 512 : kc * CG + 768],
                            start=(kc == 0),
                            stop=False,
                        )
                    nc.tensor.matmul(
                        ps[:], ones_row[:, :], vb_row[:], start=False, stop=True
                    )
                    vt = v_aug[:, tt * HPC * 65 : (tt + 1) * HPC * 65].rearrange(
                        "p (h c) -> p h c", c=65
                    )[:, :, 0:64]
                    nc.scalar.activation(
                        vt, ps[:].rearrange("p (h c) -> p h c", c=64),
                        mybir.ActivationFunctionType.Copy,
                    )

                def qk_chunk(i, tch):
                    ps2 = pK.tile([128, 512], F32, tag="big", bufs=5)
                    for kc in range(KC):
                        nc.tensor.matmul(
                            ps2[:],
                            wq16[:, kc * CG + i * 128 : kc * CG + (i + 1) * 128],
                            xT[:, kc * T + tch * 512 : kc * T + (tch + 1) * 512],
                            start=(kc == 0),
                            stop=(kc == KC - 1),
                        )
                    if (i + tch) % 2 == 0:
                        nc.scalar.activation(
                            qkT[:, i * T + tch * 512 : i * T + (tch + 1) * 512],
                            ps2[:],
                            mybir.ActivationFunctionType.Identity,
                            bias=qk_bias[:, i : i + 1],
                        )
                    else:
                        nc.vector.tensor_scalar_add(
                            qkT[:, i * T + tch * 512 : i * T + (tch + 1) * 512],
                            ps2[:],
                            qk_bias[:, i : i + 1],
                        )

                def attn(l, qb, feed):
                    p0 = 64 * (l % 2)
                    qT = qkT[p0 : p0 + 64, (l // 2) * T + qb * 512 : (l // 2) * T + (qb + 1) * 512]
                    kT = qkT[p0 : p0 + 64, (2 + l // 2) * T : (3 + l // 2) * T]
                    o = pK.tile([65, 512], F32, tag="ot", bufs=3)
                    ot[l] = o
                    # diagonal band first: its S->exp->mask->PV latency chain is
                    # then covered by the queued full-tile matmuls behind it
                    first = [True]

                    def flag():
                        f = first[0]
                        first[0] = False
                        return f

                    for j in range(4):
                        kj = 4 * qb + j
                        off = j * 128
                        st = pK.tile([128, 512], F32, tag="big", bufs=5)
                        nc.tensor.matmul(
                            st[:, off:512],
                            kT[:, kj * 128 : (kj + 1) * 128],
                            qT[:, off:512],
                            start=True,
                            stop=True,
                        )
                        pt = stage.tile([128, 512], F16, tag="pt", bufs=6)
                        nc.scalar.activation(
                            pt[:, off:512], st[:, off:512],
                            mybir.ActivationFunctionType.Exp,
                            scale=0.125,
                        )
                        nc.gpsimd.tensor_mul(
                            pt[:, off : off + 128], pt[:, off : off + 128], mask_t[:]
                        )
                        vv = v_aug[:, kj * HPC * 65 + l * 65 : kj * HPC * 65 + (l + 1) * 65]
                        if off + 128 < 512:
                            nc.tensor.matmul(
                                o[:, off + 128 : 512], vv, pt[:, off + 128 : 512],
                                start=flag(), stop=False,
                            )
                        nc.tensor.matmul(
                            o[:, off : off + 128], vv, pt[:, off : off + 128],
                            start=flag() if off == 0 else False,
                            stop=(qb == 0),
                        )
                        feed()
                    for kj in range(4 * qb):
                        st = pK.tile([128, 512], F32, tag="big", bufs=5)
                        nc.tensor.matmul(
                            st[:],
                            kT[:, kj * 128 : (kj + 1) * 128],
                            qT[:],
                            start=True,
                            stop=True,
                        )
                        pt = stage.tile([128, 512], F16, tag="pt", bufs=6)
                        nc.scalar.activation(
                            pt[:], st[:],
                            mybir.ActivationFunctionType.Exp,
                            scale=0.125,
                        )
                        vv = v_aug[:, kj * HPC * 65 + l * 65 : kj * HPC * 65 + (l + 1) * 65]
                        nc.tensor.matmul(
                            o[:], vv, pt[:],
                            start=False,
                            stop=(kj == 4 * qb - 1),
                        )
                        feed()

                def norm(pair, qb, bc_act=False):
                    # heads 2*pair, 2*pair+1 -> oT2[pair] cols [qb*512, +512)
                    l0, l1 = 2 * pair, 2 * pair + 1
                    bc = pK.tile([128, 512], F32, tag="big", bufs=5)
                    for k, l in enumerate((l0, l1)):
                        rs_sb = stage.tile([1, 512], F32, tag=f"rs{k}", bufs=2, name=f"rs_{k}")
                        nc.vector.tensor_copy(rs_sb[:], ot[l][64:65, :])
                        rc = stage.tile([1, 512], F32, tag=f"rc{k}", bufs=2, name=f"rc_{k}")
                        nc.vector.reciprocal_approx_fast(rc[:], rs_sb[:])
                        rc16 = stage.tile([1, 512], F16, tag=f"rc16{k}", bufs=2, name=f"rc16_{k}")
                        nc.vector.tensor_copy(rc16[:], rc[:])
                        nc.tensor.matmul(
                            bc[k * 64 : (k + 1) * 64, :],
                            ones_row[:, 0:64],
                            rc16[:],
                            start=True,
                            stop=True,
                        )
                    bc_sb = stage.tile([128, 512], F16, tag="bcs", bufs=2)
                    if bc_act:
                        nc.scalar.activation(
                            bc_sb[:], bc[:], mybir.ActivationFunctionType.Copy
                        )
                    else:
                        nc.vector.tensor_copy(bc_sb[:], bc[:])
                    dst = oT2[pair][:, qb * 512 : (qb + 1) * 512]
                    nc.vector.tensor_mul(dst[0:64, :], ot[l0][0:64, :], bc_sb[0:64, :])
                    nc.vector.tensor_mul(dst[64:128, :], ot[l1][0:64, :], bc_sb[64:128, :])

                def proj_half(qb, hf, split_dma=False):
                    part2 = stage.tile([128, 2 * C], F16, tag="part", bufs=2)
                    for jj, j in enumerate((2 * hf, 2 * hf + 1)):
                        tt = qb * 4 + j
                        for nch in range(2):
                            pp = pK.tile([128, 512], F32, tag="big", bufs=5)
                            for hp in range(2):
                                nc.tensor.matmul(
                                    pp[:],
                                    oT2[hp][:, tt * 128 : (tt + 1) * 128],
                                    wp2[hp][:, nch * 512 : (nch + 1) * 512],
                                    start=(hp == 0),
                                    stop=(hp == 1),
                                )
                            nc.vector.scalar_tensor_tensor(
                                out=part2[:, jj * C + nch * 512 : jj * C + (nch + 1) * 512],
                                in0=pp[:],
                                scalar=1.0,
                                in1=pbias_rep[:, nch * 512 : (nch + 1) * 512],
                                op0=mybir.AluOpType.mult,
                                op1=mybir.AluOpType.add,
                            )
                    tt0 = qb * 4 + 2 * hf
                    if split_dma:
                        for jj in range(2):
                            nc.sync.dma_start(
                                part_r[:, tt0 + jj : tt0 + jj + 1, :],
                                part2[:, jj * C : (jj + 1) * C].rearrange(
                                    "p (a c) -> p a c", a=1
                                ),
                            )
                    else:
                        nc.sync.dma_start(
                            part_r[:, tt0 : tt0 + 2, :],
                            part2[:].rearrange("p (a c) -> p a c", a=2),
                        )

                def rs_coll(qb):
                    nc.gpsimd.collective_compute(
                        "ReduceScatter",
                        mybir.AluOpType.add,
                        replica_groups=[[0, 1, 2, 3], [4, 5, 6, 7]],
                        ins=[partial_d[qb * 512 : (qb + 1) * 512, :]],
                        outs=[rsout_d[qb][:]],
                    )

                def out_dma(qb):
                    # on SP: also serializes collectives — partial writes for
                    # RS(qb+1) queue behind this wait on RS(qb) completion
                    nc.sync.dma_start(
                        out_part[qb * 128 : (qb + 1) * 128, :], rsout_d[qb][:]
                    )

                # ---------------- emission schedule ----------------
                def gemm_items(qb, defer_v=False):
                    items = []
                    deferred = []
                    for ci in QB_CHUNKS[qb]:
                        for half, tt in enumerate(CHUNK_TILES[ci]):
                            for g2 in range(2):
                                items.append(
                                    lambda ci=ci, half=half, tt=tt, g2=g2:
                                        transpose_granule(ci, half, tt, g2)
                                )
                    for tt in range(4 * qb, 4 * qb + 4):
                        (deferred if defer_v else items).append(
                            lambda tt=tt: v_chunk(tt)
                        )
                    for i in (0, 2, 1, 3):
                        items.append(lambda i=i, qb=qb: qk_chunk(i, qb))
                    return items, deferred

                def make_feeder(items, nsteps):
                    state = {"credit": 0.0, "idx": 0}
                    rate = len(items) / max(1, nsteps)

                    def feed():
                        state["credit"] += rate
                        while state["credit"] >= 1.0 and state["idx"] < len(items):
                            items[state["idx"]]()
                            state["idx"] += 1
                            state["credit"] -= 1.0
                    def drain():
                        while state["idx"] < len(items):
                            items[state["idx"]]()
                            state["idx"] += 1
                    return feed, drain

                for ci in QB_CHUNKS[0]:
                    transpose_chunk_items = [
                        (lambda ci=ci, half=half, tt=tt, g2=g2:
                            transpose_granule(ci, half, tt, g2))
                        for half, tt in enumerate(CHUNK_TILES[ci])
                        for g2 in range(2)
                    ]
                    for it in transpose_chunk_items:
                        it()
                bias_reps()
                for tt in range(4):
                    v_chunk(tt)
                for i in (0, 2, 1, 3):
                    qk_chunk(i, 0)
                deferred_v = []
                for qb in range(QB):
                    if qb + 1 < QB:
                        items, deferred_v = gemm_items(qb + 1, defer_v=(qb + 1 == 3))
                    else:
                        items = deferred_v
                    # deferred V items must finish within head 0's full-tile
                    # steps: attn(0)'s diagonal band reads v_aug written by them
                    nsteps = 4 * qb if qb + 1 == QB else 8 * (4 * qb + 4)
                    feed, drain = make_feeder(items, nsteps)
                    # head 0 of the last block must run its diagonal band last
                    # (deferred V chunks are fed during its full tiles); other
                    # heads take the diag band first to keep its latency chain
                    # off the critical path
                    dfirst = qb != 0
                    attn(0, qb, feed, qb not in (0, QB - 1))
                    if qb > 0:
                        norm(1, qb - 1)
                        if qb > 1:
                            out_dma(qb - 2)
                        proj_half(qb - 1, 0)
                    attn(1, qb, feed, dfirst)
                    if qb > 0:
                        proj_half(qb - 1, 1)
                        rs_coll(qb - 1)
                    attn(2, qb, feed, dfirst)
                    norm(0, qb)
                    attn(3, qb, feed, dfirst)
                    drain()
                norm(1, QB - 1, bc_act=True)
                out_dma(QB - 2)
                proj_half(QB - 1, 0)
                proj_half(QB - 1, 1, split_dma=True)
                rs_coll(QB - 1)
                out_dma(QB - 1)

                if DEBUG:
                    nc.gpsimd.dma_start(dbg["qkT"][:], qkT[:])
                    nc.gpsimd.dma_start(dbg["v_aug"][:], v_aug[:])
                    nc.gpsimd.dma_start(dbg["oT2"][:, 0:T], oT2[0][:])
                    nc.gpsimd.dma_start(dbg["oT2"][:, T : 2 * T], oT2[1][:])
                    nc.gpsimd.dma_start(dbg["partial"][:], partial_d[:])

    nc.finalize()
    return nc


_NC = None


def _get_nc():
    global _NC
    if _NC is None:
        _NC = _build()
    return _NC


def _make_in_maps(x, Wqkv, bqkv, Wproj, bproj):
    x = np.asarray(x, dtype=np.float32)
    Wqkv = np.asarray(Wqkv, dtype=np.float32)
    bqkv = np.asarray(bqkv, dtype=np.float32)
    Wproj = np.asarray(Wproj, dtype=np.float32)
    bproj = np.asarray(bproj, dtype=np.float32)
    zeros_c = np.zeros((1, C), np.float32)

    def perm_qkv(w):
        # (..., h*192 + t*64 + c) -> (..., t*256 + h*64 + c)
        s = w.shape[:-1]
        return np.ascontiguousarray(
            w.reshape(*s, HPC, 3, HD).swapaxes(-3, -2).reshape(*s, CG)
        )

    in_maps = []
    for c in range(N_CORES):
        b, g = divmod(c, 4)
        in_maps.append(
            {
                "x": np.ascontiguousarray(x[b]),
                "wqkv": perm_qkv(Wqkv[:, g * CG : (g + 1) * CG]),
                "bqkv": perm_qkv(bqkv[g * CG : (g + 1) * CG]).reshape(1, CG),
                "wproj": np.ascontiguousarray(Wproj[g * PD : (g + 1) * PD, :]),
                "bproj": bproj.reshape(1, C) if g == 0 else zeros_c,
            }
        )
    return in_maps


def _run(in_maps, trace=False):
    nc = _get_nc()
    return run_bass_kernel_spmd(nc, in_maps, list(range(N_CORES)), trace=trace)


def kernel(x, Wqkv, bqkv, Wproj, bproj):
    in_maps = _make_in_maps(x, Wqkv, bqkv, Wproj, bproj)
    res = _run(in_maps)
    out = np.empty((B, T, C), np.float32)
    for c in range(N_CORES):
        b, g = divmod(c, 4)
        op = np.asarray(res.results[c]["out_part"], dtype=np.float32)
        for qb in range(QB):
            out[b, qb * 512 + g * 128 : qb * 512 + (g + 1) * 128, :] = op[
                qb * 128 : (qb + 1) * 128
            ]
    return out
